# revision 1
# baseline (speedup 1.0000x reference)
"""Trainium2 Bass kernel for CustomYOLOLoss (N=512, S=52, NB=3), 8-core data parallel.

Layout: shard N across 8 cores (64 each). Per core the 64*52*52 = 173056 cells
are viewed as [128 partitions x 1352 cells]; each partition row holds a
contiguous DRAM span of 1352 cells x 15 (input) / 5 (target) channels, so
channel planes are strided APs into the raw SBUF tiles.

The whole per-core input (108 KB/partition) is preloaded into per-chunk SBUF
tiles with no slot reuse (DMA instructions then need zero sync waits, which
matters: the DMA pseudo-instruction has a single wait slot). Compute runs
per chunk of F cells/partition:
  ACT : sigmoid of the 12 box-coord channels; softplus(-c) of the 3 conf
        logits via exp(-c) -> ln(1+x) (natural_log_exp table set)
  DVE : IoU of each box vs target using the half-width identity
          iw_raw = (pw/2+tw/2) - |px-tx|,  ew = (pw/2+tw/2) + |px-tx|
        responsible-box selection via strict-greater masks + copy_predicated
        (matches jnp.argmax first-wins ties), GIoU of the selected box with
        reciprocal_approx_fast, and fused multiply+reduce accumulators
        (tensor_tensor_reduce) for the masked sums.
Host: tiny [128, 6*NBLK] partial-sum tensor per core; final divides in fp64.

Partial sums per chunk: S   = sum(bce0_sum)        over all cells
                        T1  = sum(obj*bce0_sum)
                        T2  = sum(obj*c_sel)
                        NO  = sum(obj*bce1_sel)      (= loss_obj numerator)
                        G   = sum(obj*giou_sel)
                        NOBJ= sum(obj)
with bce1 = softplus(-c) = bce(c,1), bce0 = c + bce1 = bce(c,0) so that
  num_noobj1 = S - T1, num_noobj2 = T1 - T2 - NO, num_bbox = NOBJ - G.
"""

import os
import numpy as np

import concourse.bass as bass
import concourse.bacc as bacc
import concourse.mybir as mybir
import concourse.tile as tile
from concourse.bass_utils import run_bass_kernel_spmd

F32 = mybir.dt.float32
U8 = mybir.dt.uint8
AF = mybir.ActivationFunctionType
ALU = mybir.AluOpType

N, S, NB = 512, 52, 3
CORES = 8
NPC = N // CORES                      # 64 images per core
P = 128
CELLS = NPC * S * S                   # 173056
X = CELLS // P                        # 1352 cells per partition
EPS = 1e-7

F = int(os.environ.get("YOLO_F", "169"))          # cells/partition per chunk
REPEAT = int(os.environ.get("YOLO_REPEAT", "1"))
NBLK = X // F
NACC = 8 if os.environ.get("YOLO_DEBUG2", "0") == "1" else 6

_nc_cache = {}


def build_nc():
    key = (F, REPEAT)
    if key in _nc_cache:
        return _nc_cache[key]
    nc = bacc.Bacc(trn_type="TRN2", target_bir_lowering=False)
    inp = nc.dram_tensor("input", [P, X * 15], F32, kind="ExternalInput")
    tgt = nc.dram_tensor("target", [P, X * 5], F32, kind="ExternalInput")
    out = nc.dram_tensor("out", [P, NACC * NBLK], F32, kind="ExternalOutput")
    debug = os.environ.get("YOLO_DEBUG", "0") == "1"
    if debug:
        dbg = nc.dram_tensor("dbg", [P, 48 * F], F32, kind="ExternalOutput")

    with tile.TileContext(nc) as tc:
        with (
            tc.tile_pool(name="dma", bufs=1) as dma_pool,
            tc.tile_pool(name="work", bufs=1) as work,
            tc.tile_pool(name="scr", bufs=2) as scr,
            tc.tile_pool(name="actout", bufs=2) as actout,
            tc.tile_pool(name="acts1", bufs=1) as acts1,
            tc.tile_pool(name="accp", bufs=1) as accp,
        ):
            acc = accp.tile([P, NACC * NBLK], F32)

            for rep in range(REPEAT):
              # --- preload the whole core's input; fresh tiles, zero DMA waits
              tins, ttgs = [], []
              for blk in range(NBLK):
                tin = dma_pool.tile([P, F * 15], F32, tag=f"tin{blk}")
                ttg = dma_pool.tile([P, F * 5], F32, tag=f"ttg{blk}")
                nc.sync.dma_start(tin[:], inp[:, blk * F * 15:(blk + 1) * F * 15])
                nc.sync.dma_start(ttg[:], tgt[:, blk * F * 5:(blk + 1) * F * 5])
                tins.append(tin)
                ttgs.append(ttg)

              for blk in range(NBLK):
                tin, ttg = tins[blk], ttgs[blk]
                # Views of the raw interleaved tiles
                tin_v = tin[:].rearrange("p (f b c) -> p f b c", b=3, c=5)
                ttg_v = ttg[:].rearrange("p (f c) -> p f c", c=5)
                coords_in = tin_v[:, :, :, 1:5]          # [(15,F),(5,3),(1,4)]
                conf_all = tin_v[:, :, :, 0]             # [(15,F),(5,3)]

                def conf_b(b, tin_v=tin_v):
                    return tin_v[:, :, b, 0]             # [(15,F)] offset 5b

                # --- ACT: sigmoid of 12 coord channels -> sig[b][c][f]
                sig = actout.tile([P, 12 * F], F32, tag="sig")
                sig_v = sig[:].rearrange("p (b c f) -> p f b c", b=3, c=4)
                nc.scalar.activation(sig_v, coords_in, AF.Sigmoid)
                # per-box plane helpers into sig: col (4b+c)*F
                sig_r = sig[:].rearrange("p (b c f) -> p c b f", b=3, c=4)

                # --- ACT: bce1_b = softplus(-c_b) = ln(1 + exp(-c_b))
                ext = acts1.tile([P, 3 * F], F32, tag=f"ext{blk % 8}")
                ext_v = ext[:].rearrange("p (b f) -> p f b", b=3)
                nc.scalar.activation(ext_v, conf_all, AF.Exp, scale=-1.0)
                bce1 = acts1.tile([P, 3 * F], F32, tag=f"bce1{blk % 8}")
                nc.scalar.activation(bce1[:], ext[:], AF.Ln, bias=1.0)

                # --- target-derived planes
                t22 = work.tile([P, 2 * F], F32, tag="t22")      # tw/2, th/2
                t22_v = t22[:].rearrange("p (c f) -> p f c", c=2)
                nc.vector.tensor_scalar(t22_v, ttg_v[:, :, 3:5], 0.5, None, ALU.mult)
                areab = work.tile([P, F], F32, tag="areab")
                nc.vector.tensor_tensor(areab[:], ttg_v[:, :, 3], ttg_v[:, :, 4],
                                        ALU.mult)
                nc.vector.tensor_scalar(areab[:], areab[:], EPS, None, ALU.add)
                obj = work.tile([P, F], F32, tag="obj")
                nc.vector.tensor_scalar(obj[:], ttg_v[:, :, 0], 0.0, None, ALU.is_gt)

                # --- geometry (classic corners, matches reference rounding)
                # pred xyxy: a1 = sig_xy - sig_wh/2 ; a2 = sig_xy + sig_wh/2
                p22 = work.tile([P, 6 * F], F32, tag="p22")      # b-major (w,h)/2
                p22_whv = p22[:].rearrange("p (b c f) -> p c b f", b=3, c=2)
                nc.vector.tensor_scalar(p22_whv, sig_r[:, 2:4, :, :], 0.5, None,
                                        ALU.mult)
                # tgt xyxy (shared): b1 = txy - t22 ; b2 = txy + t22
                bxy1 = work.tile([P, 2 * F], F32, tag="bxy1")
                bxy1_v = bxy1[:].rearrange("p (c f) -> p f c", c=2)
                nc.vector.tensor_tensor(bxy1_v, ttg_v[:, :, 1:3],
                                        t22[:].rearrange("p (c f) -> p f c", c=2),
                                        ALU.subtract)
                bxy2 = work.tile([P, 2 * F], F32, tag="bxy2")
                bxy2_v = bxy2[:].rearrange("p (c f) -> p f c", c=2)
                nc.vector.tensor_tensor(bxy2_v, ttg_v[:, :, 1:3],
                                        t22[:].rearrange("p (c f) -> p f c", c=2),
                                        ALU.add)
                axy1 = work.tile([P, 6 * F], F32, tag="axy1")
                axy2 = work.tile([P, 6 * F], F32, tag="axy2")
                for b in range(NB):
                    sxy = sig[:, (4 * b) * F:(4 * b + 2) * F]
                    pb_ = p22[:, b * 2 * F:(b + 1) * 2 * F]
                    nc.vector.tensor_tensor(axy1[:, b * 2 * F:(b + 1) * 2 * F],
                                            sxy, pb_, ALU.subtract)
                    nc.vector.tensor_tensor(axy2[:, b * 2 * F:(b + 1) * 2 * F],
                                            sxy, pb_, ALU.add)
                # lt/rb and enclosure corners; reuse axy tiles in place:
                # iwr = relu(min(a2,b2) - max(a1,b1)); ew = max(a2,b2) - min(a1,b1)
                lt = work.tile([P, 6 * F], F32, tag="lt")
                elt = work.tile([P, 6 * F], F32, tag="elt")
                for b in range(NB):
                    sl6 = slice(b * 2 * F, (b + 1) * 2 * F)
                    nc.vector.tensor_tensor(lt[:, sl6], axy1[:, sl6], bxy1[:],
                                            ALU.max)
                    nc.vector.tensor_tensor(elt[:, sl6], axy1[:, sl6], bxy1[:],
                                            ALU.min)
                    nc.vector.tensor_tensor(axy1[:, sl6], axy2[:, sl6], bxy2[:],
                                            ALU.min)      # rb (reuse axy1)
                    nc.vector.tensor_tensor(axy2[:, sl6], axy2[:, sl6], bxy2[:],
                                            ALU.max)      # erb (in place)
                iwr = work.tile([P, 6 * F], F32, tag="iwr")
                nc.vector.tensor_tensor(iwr[:], axy1[:], lt[:], ALU.subtract)
                nc.vector.tensor_scalar(iwr[:], iwr[:], 0.0, None, ALU.max)
                ew = work.tile([P, 6 * F], F32, tag="ew")
                nc.vector.tensor_tensor(ew[:], axy2[:], elt[:], ALU.subtract)

                # geo = [inter(3) | union+eps(3) | enc(3)] planes
                geo = work.tile([P, 9 * F], F32, tag="geo")
                iwr_v = iwr[:].rearrange("p (b c f) -> p c b f", b=3, c=2)
                nc.vector.tensor_tensor(
                    geo[:, 0:3 * F].rearrange("p (b f) -> p b f", b=3),
                    iwr_v[:, 0, :, :], iwr_v[:, 1, :, :], ALU.mult)
                ew_v = ew[:].rearrange("p (b c f) -> p c b f", b=3, c=2)
                nc.vector.tensor_tensor(
                    geo[:, 6 * F:9 * F].rearrange("p (b f) -> p b f", b=3),
                    ew_v[:, 0, :, :], ew_v[:, 1, :, :], ALU.mult)
                # area_a = pw*ph ; s = area_a + (area_b+eps) ; UE = s - inter
                aa = work.tile([P, 3 * F], F32, tag="aa")
                nc.vector.tensor_tensor(
                    aa[:].rearrange("p (b f) -> p b f", b=3),
                    sig_r[:, 2, :, :], sig_r[:, 3, :, :], ALU.mult)
                for b in range(NB):
                    nc.vector.tensor_tensor(geo[:, (3 + b) * F:(4 + b) * F],
                                            aa[:, b * F:(b + 1) * F], areab[:],
                                            ALU.add)
                nc.vector.tensor_tensor(geo[:, 3 * F:6 * F], geo[:, 3 * F:6 * F],
                                        geo[:, 0:3 * F], ALU.subtract)
                # iou_b = inter_b / UE_b
                rue = work.tile([P, 3 * F], F32, tag="rue")
                nc.vector.reciprocal_approx_fast(rue[:], geo[:, 3 * F:6 * F])
                nc.vector.tensor_tensor(geo[:, 0:3 * F], geo[:, 0:3 * F], rue[:],
                                        ALU.mult)

                # --- responsible-box masks (strict greater = argmax first-wins)
                iou0, iou1, iou2 = (geo[:, b * F:(b + 1) * F] for b in range(3))
                mk = work.tile([P, 2 * F], U8, tag="mk")         # m01 | m2
                mxt = work.tile([P, F], F32, tag="mxt")
                nc.vector.tensor_tensor(mk[:, 0:F], iou1, iou0, ALU.is_gt)
                nc.vector.tensor_tensor(mxt[:], iou0, iou1, ALU.max)
                nc.vector.tensor_tensor(mk[:, F:2 * F], iou2, mxt[:], ALU.is_gt)
                # --- select (iou, UE, enc) of the responsible box
                sel3 = work.tile([P, 3 * F], F32, tag="sel3")
                for q in range(3):
                    sq = sel3[:, q * F:(q + 1) * F]
                    base = q * 3 * F
                    nc.vector.tensor_copy(sq, geo[:, base:base + F])
                    nc.vector.copy_predicated(sq, mk[:, 0:F],
                                              geo[:, base + F:base + 2 * F])
                    nc.vector.copy_predicated(sq, mk[:, F:2 * F],
                                              geo[:, base + 2 * F:base + 3 * F])

                # --- GIoU of selected box: giou = iou - (enc-union)/(enc+eps)
                ee = work.tile([P, F], F32, tag="ee")
                nc.vector.tensor_scalar(ee[:], sel3[:, 2 * F:3 * F], EPS, None,
                                        ALU.add)
                dd = work.tile([P, F], F32, tag="dd")
                nc.vector.tensor_tensor(dd[:], ee[:], sel3[:, F:2 * F],
                                        ALU.subtract)
                ree = work.tile([P, F], F32, tag="ree")
                nc.vector.reciprocal_approx_fast(ree[:], ee[:])
                tt = work.tile([P, F], F32, tag="tt")
                nc.vector.tensor_tensor(tt[:], dd[:], ree[:], ALU.mult)
                nc.vector.tensor_tensor(dd[:], sel3[:, 0:F], tt[:], ALU.subtract)

                # --- selected conf logit and bce1
                csel = work.tile([P, F], F32, tag="csel")
                nc.vector.tensor_copy(csel[:], conf_b(0))
                nc.vector.copy_predicated(csel[:], mk[:, 0:F], conf_b(1))
                nc.vector.copy_predicated(csel[:], mk[:, F:2 * F], conf_b(2))
                bsel = work.tile([P, F], F32, tag="bsel")
                nc.vector.tensor_copy(bsel[:], bce1[:, 0:F])
                nc.vector.copy_predicated(bsel[:], mk[:, 0:F], bce1[:, F:2 * F])
                nc.vector.copy_predicated(bsel[:], mk[:, F:2 * F],
                                          bce1[:, 2 * F:3 * F])

                # --- bce0_sum = sum_b c_b + sum_b bce1_b
                cs = work.tile([P, F], F32, tag="cs")
                nc.vector.tensor_tensor(cs[:], conf_b(0), conf_b(1), ALU.add)
                nc.vector.tensor_tensor(cs[:], cs[:], conf_b(2), ALU.add)
                bs = work.tile([P, F], F32, tag="bs")
                nc.vector.tensor_tensor(bs[:], bce1[:, 0:F], bce1[:, F:2 * F],
                                        ALU.add)
                nc.vector.tensor_tensor(bs[:], bs[:], bce1[:, 2 * F:3 * F],
                                        ALU.add)
                nc.vector.tensor_tensor(cs[:], cs[:], bs[:], ALU.add)

                # --- fused masked sums into acc columns
                col = blk * NACC

                def acccol(i, col=col):
                    return acc[:, col + i:col + i + 1]

                nc.vector.tensor_reduce(acccol(0), cs[:], mybir.AxisListType.X,
                                        ALU.add)
                extra = (((6, sel3[:, 0:F]), (7, tt[:]))
                         if NACC == 8 else ())
                for i, val in ((1, cs[:]), (2, csel[:]), (3, bsel[:]), (4, dd[:])) + extra:
                    sc = scr.tile([P, F], F32, tag="ttr_scr")
                    nc.vector.tensor_tensor(sc[:], obj[:], val, ALU.mult)
                    nc.scalar.activation(sc[:], sc[:], AF.Copy,
                                         accum_out=acccol(i))
                nc.vector.tensor_reduce(acccol(5), obj[:], mybir.AxisListType.X,
                                        ALU.add)

                if debug and blk == 0:
                    nc.sync.dma_start(dbg[:, 0:9 * F], geo[:])
                    nc.sync.dma_start(dbg[:, 9 * F:12 * F], sel3[:])
                    nc.sync.dma_start(dbg[:, 12 * F:13 * F], dd[:])
                    nc.sync.dma_start(dbg[:, 13 * F:14 * F], ee[:])
                    mkf = work.tile([P, 2 * F], F32, tag="mkf")
                    nc.vector.tensor_copy(mkf[:], mk[:])
                    nc.sync.dma_start(dbg[:, 14 * F:16 * F], mkf[:])
                    nc.sync.dma_start(dbg[:, 16 * F:28 * F], sig[:])
                    nc.sync.dma_start(dbg[:, 28 * F:34 * F], axy1[:])
                    nc.sync.dma_start(dbg[:, 34 * F:40 * F], iwr[:])
                    nc.sync.dma_start(dbg[:, 40 * F:46 * F], ew[:])
                    nc.sync.dma_start(dbg[:, 46 * F:48 * F], t22[:])

            nc.gpsimd.dma_start(out[:], acc[:])

    nc.compile()
    _nc_cache[key] = nc
    return nc


def kernel(input, target):
    nc = build_nc()
    in_maps = []
    for c in range(CORES):
        sl = slice(c * NPC, (c + 1) * NPC)
        in_maps.append({
            "input": np.ascontiguousarray(input[sl]).reshape(P, X * 15),
            "target": np.ascontiguousarray(target[sl]).reshape(P, X * 5),
        })
    res = run_bass_kernel_spmd(nc, in_maps, core_ids=list(range(CORES)))
    total = np.zeros(NACC, dtype=np.float64)
    for r in res.results:
        total += r["out"].reshape(P, NBLK, NACC).sum(axis=(0, 1), dtype=np.float64)
    S_all, T1, T2, NO, G, NOBJ = total
    n_obj = NOBJ
    n_noobj = float(N * S * S) - n_obj
    num1 = S_all - T1
    num2 = T1 - T2 - NO
    num_bbox = n_obj - G
    loss_noobj = num1 / (n_noobj * NB) + num2 / (n_obj * (NB - 1))
    loss_bbox = num_bbox / n_obj
    loss_obj = NO / n_obj
    loss = loss_obj + loss_bbox + loss_noobj
    return (np.float32(loss), np.float32(loss_noobj), np.float32(loss_bbox),
            np.float32(loss_obj))

